# revision 1
# baseline (speedup 1.0000x reference)
"""AsymmetricSVD segment-reduce kernel for 8 TRN2 NeuronCores.

Strategy (data-parallel over segments):
  - Core m owns segments [512m, 512(m+1)) and their contiguous implicit
    entries (segment_ids is sorted).
  - Host precomputes per-entry scalar a_e = r_e - MU - bu[user[seg_e]] and a
    fused bf16 table XY = [X | Y - bi*X] (so w*X + Y == a*X + Y').
  - Device gathers 512B rows of XY per entry via gpsimd.dma_gather
    (hardware SWDGE gather; int16 indices -> 4 item-range buckets of 25000
    rows each), builds near-one-hot segment matrices from iota-vs-segoff
    compares, and accumulates segment sums in PSUM with PE matmuls:
        PSUM[seg, 0:128]   += sum_e a_e * X_e      (lhsT = one_hot * a)
        PSUM[seg, 128:256] += sum_e (Y' )_e        (lhsT = one_hot)
  - Epilogue: rui[seg] = bui[seg] + reduce_add(PSUM[seg, 0:256] * [Qn|Qn])
    with Qn = Q[item]*norm precomputed on host.

The Bass graph is traced per call (uniform across the 8 cores; only tensor
values differ per core), compiled, and run via run_bass_kernel_spmd.
"""

import numpy as np
import ml_dtypes

MU = 3.5
B = 4096
F = 128
NUM_ITEMS = 100000
N_CORES = 8
SEGS_PER_CORE = B // N_CORES            # 512
N_BUCKETS = 4
BUCKET_ROWS = (NUM_ITEMS + N_BUCKETS - 1) // N_BUCKETS   # 25000 < 32768 (int16)
CHUNK = 3584                             # entries per dma_gather call
PAD_SEG = -(10 ** 6)                     # int sentinel for padding entries

def _host_prep(bu, bi, Q, X, Y, user, item, imp_items, imp_ratings, segment_ids):
    """All index/scalar preprocessing. Returns per-core device arrays and
    group metadata for codegen."""
    T = imp_items.shape[0]

    # per-entry scalar (weight minus the per-item part, which is folded into Y')
    a_full = imp_ratings.astype(np.float32) - MU - bu[user[segment_ids], 0]
    Yp = Y - bi * X                                   # [NUM_ITEMS, F]
    XY = np.concatenate([X, Yp], axis=1).astype(ml_dtypes.bfloat16)  # [NI, 256]

    counts = np.bincount(segment_ids, minlength=B).astype(np.float32)
    norm = np.where(counts > 0, counts, 1.0) ** -0.5
    bui = (MU + bu[user, 0] + bi[item, 0]).astype(np.float32)        # [B]
    Qn = (Q[item] * norm[:, None]).astype(np.float32)                # [B, F]
    Qn2 = np.concatenate([Qn, Qn], axis=1)                           # [B, 256]

    # --- shard entries by segment block; bucket-stable-sort by item range ---
    bounds = np.searchsorted(segment_ids, np.arange(0, B + 1, SEGS_PER_CORE))
    cores = []
    for m in range(N_CORES):
        lo, hi = bounds[m], bounds[m + 1]
        it = imp_items[lo:hi]
        sl = (segment_ids[lo:hi] - m * SEGS_PER_CORE).astype(np.int64)
        av = a_full[lo:hi]
        bk = it // BUCKET_ROWS
        order = np.argsort(bk, kind="stable")
        it, sl, av, bk = it[order], sl[order], av[order], bk[order]
        bcnt = np.bincount(bk, minlength=N_BUCKETS)
        cores.append((it, sl, av, bcnt))

    cap = np.zeros(N_BUCKETS, np.int64)
    for m in range(N_CORES):
        cap = np.maximum(cap, cores[m][3])
    cap = ((cap + 127) // 128) * 128                    # per-bucket capacity
    offs = np.concatenate([[0], np.cumsum(cap)])
    E_pad = int(offs[-1])
    G = E_pad // 128

    # padded per-core streams
    lidx = np.zeros((N_CORES, E_pad), np.int16)          # local row in bucket
    segl = np.full((N_CORES, E_pad), PAD_SEG, np.int64)  # local segment id
    aval = np.zeros((N_CORES, E_pad), np.float32)
    for m in range(N_CORES):
        it, sl, av, bcnt = cores[m]
        pos = 0
        for b in range(N_BUCKETS):
            n = int(bcnt[b])
            d = int(offs[b])
            lidx[m, d:d + n] = (it[pos:pos + n] - b * BUCKET_ROWS).astype(np.int16)
            segl[m, d:d + n] = sl[pos:pos + n]
            aval[m, d:d + n] = av[pos:pos + n]
            pos += n

    # --- group metadata (cross-core, uniform) ---
    sg = segl.reshape(N_CORES, G, 128)
    real = sg != PAD_SEG
    any_real = real.any(axis=(0, 2))                     # [G]
    lo_g = np.where(real, sg, 10 ** 9).min(axis=(0, 2))
    hi_g = np.where(real, sg, -1).max(axis=(0, 2))
    A_g = np.where(any_real, 64 * (np.minimum(lo_g, 10 ** 9 - 1) // 64), 0)
    offmax = np.where(any_real, hi_g - A_g, 0)
    mwin = int(max(80, ((offmax.max() + 16) // 16) * 16)) if any_real.any() else 80
    if mwin > 192:
        raise RuntimeError(f"pathological segment distribution: mwin={mwin}")

    # device segoff values
    segoff = np.where(
        real, sg - A_g[None, :, None], -1000
    ).astype(ml_dtypes.bfloat16)                          # [N_CORES, G, 128]

    # chunk list: (start_entry, n_entries, bucket) — near-equal chunks per
    # bucket (multiples of 128) to avoid undersized tail calls
    chunks = []
    for b in range(N_BUCKETS):
        s, e = int(offs[b]), int(offs[b + 1])
        total_g = (e - s) // 128
        if total_g == 0:
            continue
        ncalls = max(1, (total_g * 128 + CHUNK - 1) // CHUNK)
        base_g, extra = divmod(total_g, ncalls)
        for c in range(ncalls):
            g = base_g + (1 if c < extra else 0)
            n = g * 128
            chunks.append((s, n, b))
            s += n

    meta = dict(
        E_pad=E_pad, G=G, mwin=mwin, chunks=chunks,
        A=A_g.astype(np.int64), any_real=any_real,
        offmax=offmax.astype(np.int64),
    )

    # --- device arrays per core ---
    def wrap16(x):   # entry e -> [e%16, e//16], replicated to 128 partitions
        w = x.reshape(-1, 16).T
        return np.ascontiguousarray(np.tile(w, (8, 1)))

    def wrap128(x):  # entry e -> [e%128, e//128]
        return np.ascontiguousarray(x.reshape(-1, 128).T)

    iota = np.broadcast_to(
        np.arange(mwin, dtype=np.float32), (128, mwin)
    ).astype(ml_dtypes.bfloat16)

    in_maps = []
    for m in range(N_CORES):
        in_maps.append({
            "xy": XY,
            "iota": np.ascontiguousarray(iota),
            "idx16": wrap16(lidx[m]),
            "segoff": wrap128_bf(segoff[m]),
            "aw": wrap128(aval[m]).astype(ml_dtypes.bfloat16),
            "qn2": np.ascontiguousarray(Qn2[m * SEGS_PER_CORE:(m + 1) * SEGS_PER_CORE]),
            "bui": np.ascontiguousarray(bui[m * SEGS_PER_CORE:(m + 1) * SEGS_PER_CORE]),
        })
    return in_maps, meta


def wrap128_bf(x):
    return np.ascontiguousarray(np.asarray(x).reshape(-1, 128).T)


def _build_graph(meta, stage=4):
    # stage: 1=gathers only, 2=+S build, 3=+matmuls, 4=full epilogue
    from concourse import bacc, mybir
    from concourse.tile import TileContext

    E_pad, G, mwin = meta["E_pad"], meta["G"], meta["mwin"]
    chunks = meta["chunks"]
    A, any_real, offmax = meta["A"], meta["any_real"], meta["offmax"]

    nc = bacc.Bacc("TRN2", target_bir_lowering=False, debug=False,
                   num_devices=N_CORES, num_swdge_queues=4)
    bf16, f32, i16 = mybir.dt.bfloat16, mybir.dt.float32, mybir.dt.int16

    xy_d = nc.declare_dram_parameter("xy", [NUM_ITEMS, 256], bf16, isOutput=False)
    iota_d = nc.declare_dram_parameter("iota", [128, mwin], bf16, isOutput=False)
    idx_d = nc.declare_dram_parameter("idx16", [128, E_pad // 16], i16, isOutput=False)
    seg_d = nc.declare_dram_parameter("segoff", [128, G], bf16, isOutput=False)
    aw_d = nc.declare_dram_parameter("aw", [128, G], bf16, isOutput=False)
    qn_d = nc.declare_dram_parameter("qn2", [SEGS_PER_CORE, 256], f32, isOutput=False)
    bui_d = nc.declare_dram_parameter("bui", [SEGS_PER_CORE], f32, isOutput=False)
    out_d = nc.declare_dram_parameter("out", [SEGS_PER_CORE], f32, isOutput=True)

    n_banks = SEGS_PER_CORE // 128  # 4

    with TileContext(nc) as tc:
        with (
            tc.tile_pool(name="const", bufs=1) as cpool,
            tc.tile_pool(name="xy", bufs=6) as xypool,
            tc.tile_pool(name="meta", bufs=3) as mpool,
            tc.tile_pool(name="sel", bufs=4) as spool,
            tc.tile_pool(name="epi", bufs=2) as epool,
            tc.tile_pool(name="psum", bufs=1, space="PSUM") as ppool,
        ):
            # per-bucket idx tiles so the first gather doesn't wait for the
            # whole preload
            bucket_bounds = []
            b0 = 0
            for b in range(N_BUCKETS):
                bn = sum(n for (s, n, bb) in chunks if bb == b)
                bucket_bounds.append((b0, bn))
                b0 += bn
            idx_tiles = []
            for b, (boff, bn) in enumerate(bucket_bounds):
                if bn == 0:
                    idx_tiles.append(None)
                    continue
                t = cpool.tile([128, bn // 16], i16, tag=f"idx{b}")
                nc.sync.dma_start(
                    out=t[:], in_=idx_d[:, boff // 16:(boff + bn) // 16])
                idx_tiles.append(t)
            # non-critical preloads on the scalar engine's HWDGE queue so the
            # sync queue only carries the gather-gating idx tiles
            iota_t = cpool.tile([128, mwin], bf16, tag="iota")
            nc.scalar.dma_start(out=iota_t[:], in_=iota_d[:])
            zeros_t = cpool.tile([128, 512], bf16, tag="zeros")
            nc.vector.memset(zeros_t[:], 0.0)
            seg_t = cpool.tile([128, G], bf16, tag="segoff")
            nc.scalar.dma_start(out=seg_t[:], in_=seg_d[:])
            aw_t = cpool.tile([128, G], bf16, tag="aw")
            nc.scalar.dma_start(out=aw_t[:], in_=aw_d[:])

            psum_t = []
            for k in range(n_banks):
                pt = ppool.tile([128, 512], f32, tag=f"bank{k}")
                psum_t.append(pt)
                nc.tensor.matmul(
                    out=pt[:, 0:512], lhsT=zeros_t[:, 0:128],
                    rhs=zeros_t[:, 0:512], start=True, stop=False,
                )

            for ci, (start, n, b) in enumerate(chunks):
                if stage < 1:
                    break
                nG = n // 128
                xyt = xypool.tile([128, nG, 256], bf16, tag="xyt")
                boff = bucket_bounds[b][0]
                bidx = idx_tiles[b]
                nc.gpsimd.dma_gather(
                    out_ap=xyt[:],
                    in_ap=xy_d[b * BUCKET_ROWS:(b + 1) * BUCKET_ROWS, :],
                    idxs_ap=bidx[:, (start - boff) // 16:(start - boff + n) // 16],
                    num_idxs=n,
                    num_idxs_reg=n,
                    elem_size=256,
                    single_packet=False,
                    queue_num=ci % 4,
                )
                if stage < 2:
                    continue
                c0g = start // 128
                so_t = seg_t[:, c0g:c0g + nG]
                a_t = aw_t[:, c0g:c0g + nG]
                S_t = spool.tile([128, nG, mwin], bf16, tag="S")
                Sp_t = spool.tile([128, nG, mwin], bf16, tag="Sp")
                nc.vector.tensor_tensor(
                    out=S_t[:],
                    in0=iota_t[:].unsqueeze(1).to_broadcast((128, nG, mwin)),
                    in1=so_t[:].to_broadcast((128, nG, mwin)),
                    op=mybir.AluOpType.is_equal,
                )
                nc.vector.tensor_tensor(
                    out=Sp_t[:], in0=S_t[:],
                    in1=a_t[:].to_broadcast((128, nG, mwin)),
                    op=mybir.AluOpType.mult,
                )

                if stage < 3:
                    continue
                for u in range(nG):
                    g = start // 128 + u
                    if not any_real[g]:
                        continue
                    subs = [s for s in range(3)
                            if s == 0 or offmax[g] >= 64 * s]
                    for s in subs:
                        base = int(A[g]) + 64 * s
                        bank, p0 = base // 128, base % 128
                        w = min(64, mwin - 64 * s)
                        if w <= 0 or base >= SEGS_PER_CORE:
                            continue
                        for which, lhs in ((0, Sp_t), (1, S_t)):
                            c0, c1 = 128 * which, 128 * (which + 1)
                            nc.tensor.matmul(
                                out=psum_t[bank][p0:p0 + w, c0:c1],
                                lhsT=lhs[:, u, 64 * s:64 * s + w],
                                rhs=xyt[:, u, c0:c1],
                                start=False,
                                stop=False,
                            )

            # close accumulation groups (full-width, required before reads)
            for k in range(n_banks):
                nc.tensor.matmul(
                    out=psum_t[k][:, 0:512], lhsT=zeros_t[:, 0:128],
                    rhs=zeros_t[:, 0:512], start=False, stop=True,
                )

            # epilogue
            for k in range(n_banks):
                bui_t = epool.tile([128, 1], f32, tag="bui")
                nc.scalar.dma_start(out=bui_t[:], in_=bui_d[128 * k:128 * (k + 1)])
                red_t = epool.tile([128, 1], f32, tag="red")
                if stage >= 4:
                    qn_t = epool.tile([128, 256], f32, tag="qn")
                    nc.scalar.dma_start(out=qn_t[:], in_=qn_d[128 * k:128 * (k + 1), :])
                    prod_t = epool.tile([128, 256], f32, tag="prod")
                    nc.vector.tensor_tensor(
                        out=prod_t[:], in0=psum_t[k][:, 0:256], in1=qn_t[:],
                        op=mybir.AluOpType.mult,
                    )
                    nc.vector.tensor_reduce(
                        out=red_t[:, 0:1], in_=prod_t[:],
                        axis=mybir.AxisListType.X,
                        op=mybir.AluOpType.add,
                    )
                    nc.vector.tensor_add(red_t[:, 0:1], red_t[:, 0:1], bui_t[:, 0:1])
                else:
                    nc.vector.tensor_copy(out=red_t[:], in_=bui_t[:])
                nc.sync.dma_start(out=out_d[128 * k:128 * (k + 1)], in_=red_t[:, 0:1])

    nc.compile()
    return nc


def kernel(bu, bi, Q, X, Y, user, item, imp_items, imp_ratings, segment_ids,
           _sim=False, _stage=4):
    bu = np.asarray(bu, np.float32)
    bi = np.asarray(bi, np.float32)
    Q = np.asarray(Q, np.float32)
    X = np.asarray(X, np.float32)
    Y = np.asarray(Y, np.float32)
    user = np.asarray(user).astype(np.int64)
    item = np.asarray(item).astype(np.int64)
    imp_items = np.asarray(imp_items).astype(np.int64)
    imp_ratings = np.asarray(imp_ratings).astype(np.int64)
    segment_ids = np.asarray(segment_ids).astype(np.int64)

    in_maps, meta = _host_prep(bu, bi, Q, X, Y, user, item, imp_items,
                               imp_ratings, segment_ids)
    nc = _build_graph(meta, stage=_stage)

    if _sim:
        from concourse import bass_interp
        sim = bass_interp.CoreSim(nc)
        sim.assign_tensors(in_maps[0])
        sim.simulate()
        out0 = np.array(sim.tensor("out"))
        return sim, out0, in_maps, meta

    from concourse.bass_utils import run_bass_kernel_spmd
    res = run_bass_kernel_spmd(nc, in_maps, core_ids=list(range(N_CORES)),
                               trace=False)
    out = np.concatenate([res.results[m]["out"] for m in range(N_CORES)])
    return out.astype(np.float32)

